# revision 9
# baseline (speedup 1.0000x reference)
"""Additive attention (Bahdanau) on 8 TRN2 NeuronCores.

Full-problem shapes: query [4,512,512], key/value [4,512,512],
Wq/Wk [512,256], bq/bk [256], wv [256], bv [].

  q = query @ Wq + bq                       # [B,Q,H]
  k = key @ Wk + bk                         # [B,K,H]
  score[b,q,k] = wv . tanh(q[b,q]+k[b,k])   # (+bv, dropped: softmax-invariant)
  attn = softmax(score, axis=-1)
  context = attn @ value

Sharding: data-parallel over (batch, query-half): core c handles batch c//2,
query rows (c%2)*256:(c%2+1)*256. Each core sees its full key/value batch, so
softmax is core-local; gather is pure numpy concatenation.

Per-core kernel layout: h (hidden) on partitions. For each query row r, the
scalar engine computes feat = tanh(kTp + qTp[:, r]) as one [128h, 512k]
activation per h-chunk (per-partition bias = q values), the tensor engine then
contracts with wv (feat stationary, wv the 1-column moving operand) writing a
scoreT[k-partition, r] column into PSUM. Softmax runs on the transposed scores
without any max-subtraction (|score| <= sum|wv| ~ 13, safe in fp32): exp on
the scalar engine, key-sum via ones-vector matmul over partitions, reciprocal
broadcast across partitions by a stride-0 DMA. The normalized attnT is
directly the lhsT of the context matmul. attnT is un-transposed on the host.
"""

import numpy as np

import concourse.bass as bass
import concourse.tile as tile
from concourse import bacc, mybir
from concourse.bass_utils import run_bass_kernel_spmd
from concourse.masks import make_identity

F32 = mybir.dt.float32
F16 = mybir.dt.float16

P = 128          # partitions
D = 512          # DQ = DK (projection input dim)
H = 256          # hidden dim; HC = H // P h-chunks
K = 512          # keys per batch; KC = K // P key chunks
QS = 256         # query rows per core
DV = 512         # value dim
HC, KC, DC, QT = H // P, K // P, D // P, QS // P

N_CORES = 8
B, Q = 4, 512


def _build_tile_kernel(tc, ins, outs, n_rows=QS):
    nc = tc.nc
    query, key, value, Wq, bq, Wk, bk, wv = ins
    ctx_out, attnT_out = outs

    with tc.tile_pool(name="const", bufs=1) as const, \
         tc.tile_pool(name="proj", bufs=1) as proj, \
         tc.tile_pool(name="feat", bufs=6) as featp, \
         tc.tile_pool(name="tailp", bufs=1) as tailp, \
         tc.tile_pool(name="outp", bufs=2) as outp:

        # ---- input DMAs -------------------------------------------------
        q_raw = const.tile([P, QT, D], F32)
        nc.sync.dma_start(q_raw[:], query.rearrange("(t p) d -> p t d", p=P))
        k_raw = const.tile([P, KC, D], F32)
        nc.sync.dma_start(k_raw[:], key.rearrange("(t p) d -> p t d", p=P))
        v_sb = const.tile([P, KC, DV], F32)
        nc.sync.dma_start(v_sb[:], value.rearrange("(c p) v -> p c v", p=P))
        wq_sb = const.tile([P, DC, H], F32)
        nc.sync.dma_start(wq_sb[:], Wq.rearrange("(c p) h -> p c h", p=P))
        wk_sb = const.tile([P, DC, H], F32)
        nc.sync.dma_start(wk_sb[:], Wk.rearrange("(c p) h -> p c h", p=P))
        bq_sb = const.tile([P, HC], F32)
        nc.sync.dma_start(bq_sb[:], bq.rearrange("(o p) -> p o", p=P))
        bk_sb = const.tile([P, HC], F32)
        nc.sync.dma_start(bk_sb[:], bk.rearrange("(o p) -> p o", p=P))
        wv32 = const.tile([P, HC], F32)
        nc.sync.dma_start(wv32[:], wv.rearrange("(o p) -> p o", p=P))

        wv16 = const.tile([P, HC], F16)
        nc.vector.tensor_copy(wv16[:], wv32[:])
        ones_sb = const.tile([P, 1], F32)
        nc.vector.memset(ones_sb[:], 1.0)
        ident = const.tile([P, P], F32)
        make_identity(nc, ident[:])

        # ---- transpose query/key so d sits on partitions ---------------
        qT = proj.tile([P, DC, QS], F32)      # [d_inner, d_chunk, q]
        kT = proj.tile([P, DC, K], F32)
        qTp = proj.tile([P, HC, QS], F32)
        kTp = proj.tile([P, HC, K], F32)
        with tc.tile_pool(name="ps_mm", bufs=2, space="PSUM") as ps_mm:
            for t in range(QT):
                for c in range(DC):
                    pst = ps_mm.tile([P, P], F32, tag="tp")
                    nc.tensor.transpose(pst[:], q_raw[:, t, c * P:(c + 1) * P],
                                        ident[:])
                    nc.vector.tensor_copy(qT[:, c, t * P:(t + 1) * P], pst[:])
            for t in range(KC):
                for c in range(DC):
                    pst = ps_mm.tile([P, P], F32, tag="tp")
                    nc.tensor.transpose(pst[:], k_raw[:, t, c * P:(c + 1) * P],
                                        ident[:])
                    nc.vector.tensor_copy(kT[:, c, t * P:(t + 1) * P], pst[:])

            # ---- projections, already transposed: [h, q] and [h, k] ----
            for hs in range(HC):
                psq = ps_mm.tile([P, QS], F32, tag="psq", bufs=1)
                for c in range(DC):
                    nc.tensor.matmul(psq[:], wq_sb[:, c, hs * P:(hs + 1) * P],
                                     qT[:, c, :], start=(c == 0), stop=(c == DC - 1))
                nc.vector.tensor_scalar_add(qTp[:, hs, :], psq[:],
                                            bq_sb[:, hs:hs + 1])
                psk = ps_mm.tile([P, K], F32, tag="psk", bufs=1)
                for c in range(DC):
                    nc.tensor.matmul(psk[:], wk_sb[:, c, hs * P:(hs + 1) * P],
                                     kT[:, c, :], start=(c == 0), stop=(c == DC - 1))
                nc.vector.tensor_scalar_add(kTp[:, hs, :], psk[:],
                                            bk_sb[:, hs:hs + 1])

        # ---- main loop: tanh features + wv contraction ------------------
        with tc.tile_pool(name="ps_score", bufs=1, space="PSUM") as ps_score, \
             tc.tile_pool(name="ps_tail", bufs=1, space="PSUM") as ps_tail:
            score_ps = [ps_score.tile([P, QS], F32, name=f"score_{kc}")
                        for kc in range(KC)]
            for r in range(n_rows):
                feats = []
                for hs in range(HC):
                    f = featp.tile([P, K], F16, tag="feat")
                    nc.scalar.activation(f[:], kTp[:, hs, :],
                                         mybir.ActivationFunctionType.Tanh,
                                         bias=qTp[:, hs, r:r + 1])
                    feats.append(f)
                for kc in range(KC):
                    for hs in range(HC):
                        nc.tensor.matmul(score_ps[kc][:, r:r + 1],
                                         feats[hs][:, kc * P:(kc + 1) * P],
                                         wv16[:, hs:hs + 1],
                                         start=(hs == 0), stop=(hs == HC - 1))

            # ---- softmax on transposed scores ---------------------------
            expT = tailp.tile([P, KC, QS], F32)
            for kc in range(KC):
                nc.scalar.activation(expT[:, kc, :], score_ps[kc][:],
                                     mybir.ActivationFunctionType.Exp)
            sums_ps = ps_tail.tile([P, QS], F32, tag="sums")
            for kc in range(KC):
                nc.tensor.matmul(sums_ps[0:1, :], ones_sb[:], expT[:, kc, :],
                                 start=(kc == 0), stop=(kc == KC - 1))
            recip = tailp.tile([1, QS], F32)
            nc.vector.reciprocal(recip[:], sums_ps[0:1, :])
            recipB = tailp.tile([P, QS], F32)
            nc.gpsimd.partition_broadcast(recipB[:], recip[:])
            attnT = tailp.tile([P, KC, QS], F32)
            nc.vector.tensor_tensor(attnT[:], expT[:],
                                    recipB[:, None, :].to_broadcast((P, KC, QS)),
                                    mybir.AluOpType.mult)
            nc.sync.dma_start(attnT_out.rearrange("(c p) q -> p c q", p=P),
                              attnT[:])

            # ---- context = attn @ value --------------------------------
            for t in range(QT):
                psc = ps_tail.tile([P, DV], F32, tag="ctx", bufs=2)
                for kc in range(KC):
                    nc.tensor.matmul(psc[:], attnT[:, kc, t * P:(t + 1) * P],
                                     v_sb[:, kc, :],
                                     start=(kc == 0), stop=(kc == KC - 1))
                ctx_sb = outp.tile([P, DV], F32, tag="ctx_sb")
                nc.vector.tensor_copy(ctx_sb[:], psc[:])
                nc.sync.dma_start(
                    ctx_out.rearrange("(t p) v -> p t v", p=P)[:, t, :],
                    ctx_sb[:])


def build_nc(n_rows=QS):
    nc = bacc.Bacc("TRN2", target_bir_lowering=False, debug=False)
    ins = [
        nc.dram_tensor("query", [QS, D], F32, kind="ExternalInput").ap(),
        nc.dram_tensor("key", [K, D], F32, kind="ExternalInput").ap(),
        nc.dram_tensor("value", [K, DV], F32, kind="ExternalInput").ap(),
        nc.dram_tensor("Wq", [D, H], F32, kind="ExternalInput").ap(),
        nc.dram_tensor("bq", [H], F32, kind="ExternalInput").ap(),
        nc.dram_tensor("Wk", [D, H], F32, kind="ExternalInput").ap(),
        nc.dram_tensor("bk", [H], F32, kind="ExternalInput").ap(),
        nc.dram_tensor("wv", [H], F32, kind="ExternalInput").ap(),
    ]
    outs = [
        nc.dram_tensor("context", [QS, DV], F32, kind="ExternalOutput").ap(),
        nc.dram_tensor("attnT", [K, QS], F32, kind="ExternalOutput").ap(),
    ]
    with tile.TileContext(nc) as tc:
        _build_tile_kernel(tc, ins, outs, n_rows=n_rows)
    nc.compile()
    return nc


_NC_CACHE = None


def _get_nc():
    global _NC_CACHE
    if _NC_CACHE is None:
        _NC_CACHE = build_nc()
    return _NC_CACHE


def make_in_maps(query, key, value, Wq, bq, Wk, bk, wv):
    in_maps = []
    for c in range(N_CORES):
        b, half = c // 2, c % 2
        in_maps.append({
            "query": np.ascontiguousarray(query[b, half * QS:(half + 1) * QS, :]),
            "key": np.ascontiguousarray(key[b]),
            "value": np.ascontiguousarray(value[b]),
            "Wq": np.ascontiguousarray(Wq),
            "bq": np.ascontiguousarray(bq),
            "Wk": np.ascontiguousarray(Wk),
            "bk": np.ascontiguousarray(bk),
            "wv": np.ascontiguousarray(wv),
        })
    return in_maps


def gather_results(results):
    context = np.empty((B, Q, DV), np.float32)
    attn = np.empty((B, Q, K), np.float32)
    for c, r in enumerate(results):
        b, half = c // 2, c % 2
        context[b, half * QS:(half + 1) * QS, :] = r["context"]
        attn[b, half * QS:(half + 1) * QS, :] = np.ascontiguousarray(r["attnT"].T)
    return context, attn


def kernel(query, key, value, Wq, bq, Wk, bk, wv, bv, **run_kwargs):
    nc = _get_nc()
    in_maps = make_in_maps(
        np.asarray(query, np.float32), np.asarray(key, np.float32),
        np.asarray(value, np.float32), np.asarray(Wq, np.float32),
        np.asarray(bq, np.float32), np.asarray(Wk, np.float32),
        np.asarray(bk, np.float32), np.asarray(wv, np.float32))
    res = run_bass_kernel_spmd(nc, in_maps, core_ids=list(range(N_CORES)),
                               **run_kwargs)
    out = gather_results(res.results)
    if run_kwargs:
        return out, res
    return out


# revision 12
# speedup vs baseline: 1.3191x; 1.3191x over previous
"""Additive attention (Bahdanau) on 8 TRN2 NeuronCores.

Full-problem shapes: query [4,512,512], key/value [4,512,512],
Wq/Wk [512,256], bq/bk [256], wv [256], bv [].

  q = query @ Wq + bq                       # [B,Q,H]
  k = key @ Wk + bk                         # [B,K,H]
  score[b,q,k] = wv . tanh(q[b,q]+k[b,k])   # (+bv, dropped: softmax-invariant)
  attn = softmax(score, axis=-1)
  context = attn @ value

Sharding: data-parallel over (batch, query-half): core c handles batch c//2,
query rows (c%2)*256:(c%2+1)*256. Each core sees its full key/value batch, so
softmax is core-local; gather is pure numpy concatenation.

Per-core kernel layout: h (hidden) on partitions. For each query row r, the
scalar engine computes feat = tanh(kTp + qTp[:, r]) as one [128h, 512k]
activation per h-chunk (per-partition bias = q values), the tensor engine then
contracts with wv (feat stationary, wv the 1-column moving operand) writing a
scoreT[k-partition, r] column into PSUM. Softmax runs on the transposed scores
without any max-subtraction (|score| <= sum|wv| ~ 13, safe in fp32): exp on
the scalar engine, key-sum via ones-vector matmul over partitions, reciprocal
broadcast across partitions by a stride-0 DMA. The normalized attnT is
directly the lhsT of the context matmul. attnT is un-transposed on the host.
"""

import numpy as np

import concourse.bass as bass
import concourse.tile as tile
from concourse import bacc, mybir
from concourse.bass_utils import run_bass_kernel_spmd
from concourse.masks import make_identity

F32 = mybir.dt.float32
F16 = mybir.dt.float16

P = 128          # partitions
D = 512          # DQ = DK (projection input dim)
H = 256          # hidden dim; HC = H // P h-chunks
K = 512          # keys per batch; KC = K // P key chunks
QS = 256         # query rows per core
DV = 512         # value dim
HC, KC, DC, QT = H // P, K // P, D // P, QS // P

N_CORES = 8
B, Q = 4, 512


def _build_tile_kernel(tc, ins, outs, n_rows=QS):
    nc = tc.nc
    query, key, value, Wq, bq, Wk, bk, wv = ins
    ctx_out, attnT_out = outs

    with tc.tile_pool(name="const", bufs=1) as const, \
         tc.tile_pool(name="proj", bufs=1) as proj, \
         tc.tile_pool(name="feat", bufs=2) as featp, \
         tc.tile_pool(name="tailp", bufs=1) as tailp, \
         tc.tile_pool(name="outp", bufs=2) as outp:

        # ---- input DMAs (per-chunk so transposes start early) ----------
        k_raw = const.tile([P, KC, D], F32)
        key_r = key.rearrange("(t p) d -> p t d", p=P)
        for t in range(KC):
            nc.sync.dma_start(k_raw[:, t, :], key_r[:, t, :])
        q_raw = const.tile([P, QT, D], F32)
        query_r = query.rearrange("(t p) d -> p t d", p=P)
        for t in range(QT):
            nc.sync.dma_start(q_raw[:, t, :], query_r[:, t, :])
        v_sb = const.tile([P, KC, DV], F32)
        nc.sync.dma_start(v_sb[:], value.rearrange("(c p) v -> p c v", p=P))
        wq_sb = const.tile([P, DC, H], F32)
        nc.sync.dma_start(wq_sb[:], Wq.rearrange("(c p) h -> p c h", p=P))
        wk_sb = const.tile([P, DC, H], F32)
        nc.sync.dma_start(wk_sb[:], Wk.rearrange("(c p) h -> p c h", p=P))
        bq_sb = const.tile([P, HC], F32)
        nc.sync.dma_start(bq_sb[:], bq.rearrange("(o p) -> p o", p=P))
        bk_sb = const.tile([P, HC], F32)
        nc.sync.dma_start(bk_sb[:], bk.rearrange("(o p) -> p o", p=P))
        wv32 = const.tile([P, HC], F32)
        nc.sync.dma_start(wv32[:], wv.rearrange("(o p) -> p o", p=P))

        wv16 = const.tile([P, HC], F16)
        nc.vector.tensor_copy(wv16[:], wv32[:])
        ones_sb = const.tile([P, 1], F32)
        nc.vector.memset(ones_sb[:], 1.0)
        ident = const.tile([P, P], F32)
        make_identity(nc, ident[:])

        # ---- transpose query/key so d sits on partitions ---------------
        qT = proj.tile([P, DC, QS], F32)      # [d_inner, d_chunk, q]
        kT = proj.tile([P, DC, K], F32)
        qTp = proj.tile([P, HC, QS], F32)     # fp32: feeds tensor_scalar adds
        kTp16 = proj.tile([P, HC, K], F16)
        with tc.tile_pool(name="ps_mm", bufs=2, space="PSUM") as ps_mm:
            for t in range(KC):
                for c in range(DC):
                    pst = ps_mm.tile([P, P], F32, tag="tp")
                    nc.tensor.transpose(pst[:], k_raw[:, t, c * P:(c + 1) * P],
                                        ident[:])
                    nc.vector.tensor_copy(kT[:, c, t * P:(t + 1) * P], pst[:])
            for t in range(QT):
                for c in range(DC):
                    pst = ps_mm.tile([P, P], F32, tag="tp")
                    nc.tensor.transpose(pst[:], q_raw[:, t, c * P:(c + 1) * P],
                                        ident[:])
                    nc.vector.tensor_copy(qT[:, c, t * P:(t + 1) * P], pst[:])

            # ---- projections, already transposed: [h, q] and [h, k] ----
            for hs in range(HC):
                psk = ps_mm.tile([P, K], F32, tag="psk", bufs=1)
                for c in range(DC):
                    nc.tensor.matmul(psk[:], wk_sb[:, c, hs * P:(hs + 1) * P],
                                     kT[:, c, :], start=(c == 0), stop=(c == DC - 1))
                nc.vector.tensor_scalar_add(kTp16[:, hs, :], psk[:],
                                            bk_sb[:, hs:hs + 1])
                psq = ps_mm.tile([P, QS], F32, tag="psq", bufs=1)
                for c in range(DC):
                    nc.tensor.matmul(psq[:], wq_sb[:, c, hs * P:(hs + 1) * P],
                                     qT[:, c, :], start=(c == 0), stop=(c == DC - 1))
                nc.vector.tensor_scalar_add(qTp[:, hs, :], psq[:],
                                            bq_sb[:, hs:hs + 1])

        # ---- main loop: per 8-row group, DVE adds -> one big tanh ->
        #      per-row wv matvecs into transposed-score PSUM columns -----
        G = 8
        n_groups = (n_rows + G - 1) // G
        with tc.tile_pool(name="ps_score", bufs=1, space="PSUM") as ps_score, \
             tc.tile_pool(name="ps_tail", bufs=1, space="PSUM") as ps_tail, \
             tc.tile_pool(name="sump", bufs=2) as sump:
            score_ps = [ps_score.tile([P, QS], F32, name=f"score_{kc}")
                        for kc in range(KC)]
            expT = tailp.tile([P, KC, QS], F32)
            sums_ps = ps_tail.tile([P, QS], F32, tag="sums")
            attnT = tailp.tile([P, KC, QS], F32)
            recipB = tailp.tile([P, QS], F32)

            def tail_half(t):
                """softmax + context for query columns [t*P, (t+1)*P)."""
                cs = slice(t * P, (t + 1) * P)
                for kc in range(KC):
                    nc.scalar.activation(expT[:, kc, cs], score_ps[kc][:, cs],
                                         mybir.ActivationFunctionType.Exp)
                for kc in range(KC):
                    nc.tensor.matmul(sums_ps[0:1, cs], ones_sb[:],
                                     expT[:, kc, cs],
                                     start=(kc == 0), stop=(kc == KC - 1))
                sums_sb = tailp.tile([1, P], F32, tag="sums_sb", bufs=2,
                                     name="sums_sb")
                nc.vector.tensor_copy(sums_sb[:], sums_ps[0:1, cs])
                nc.gpsimd.partition_broadcast(recipB[:, cs], sums_sb[:])
                nc.vector.reciprocal(recipB[:, cs], recipB[:, cs])
                nc.vector.tensor_tensor(
                    attnT[:, :, cs], expT[:, :, cs],
                    recipB[:, None, cs].to_broadcast((P, KC, P)),
                    mybir.AluOpType.mult)
                nc.sync.dma_start(
                    attnT_out.rearrange("(c p) q -> p c q", p=P)[:, :, cs],
                    attnT[:, :, cs])
                psc = ps_tail.tile([P, DV], F32, tag="ctx", bufs=2)
                for kc in range(KC):
                    nc.tensor.matmul(psc[:], attnT[:, kc, cs], v_sb[:, kc, :],
                                     start=(kc == 0), stop=(kc == KC - 1))
                ctx_sb = outp.tile([P, DV], F32, tag="ctx_sb")
                nc.vector.tensor_copy(ctx_sb[:], psc[:])
                nc.sync.dma_start(
                    ctx_out.rearrange("(t p) v -> p t v", p=P)[:, t, :],
                    ctx_sb[:])

            for g in range(n_groups):
                rows = range(g * G, min((g + 1) * G, n_rows))
                sums = sump.tile([P, G, HC, K], F16, tag="sums")
                for j, r in enumerate(rows):
                    for hs in range(HC):
                        nc.vector.tensor_scalar_add(
                            sums[:, j, hs, :], kTp16[:, hs, :],
                            qTp[:, hs, r:r + 1])
                feat = featp.tile([P, G, HC, K], F16, tag="feat")
                nc.scalar.activation(feat[:], sums[:],
                                     mybir.ActivationFunctionType.Tanh)
                for j, r in enumerate(rows):
                    for kc in range(KC):
                        for hs in range(HC):
                            nc.tensor.matmul(
                                score_ps[kc][:, r:r + 1],
                                feat[:, j, hs, kc * P:(kc + 1) * P],
                                wv16[:, hs:hs + 1],
                                start=(hs == 0), stop=(hs == HC - 1))
                if n_rows == QS and (g + 1) * G == QS // 2:
                    tail_half(0)
            tail_half(1)
            if n_rows < QS:
                tail_half(0)


def build_nc(n_rows=QS):
    nc = bacc.Bacc("TRN2", target_bir_lowering=False, debug=False)
    ins = [
        nc.dram_tensor("query", [QS, D], F32, kind="ExternalInput").ap(),
        nc.dram_tensor("key", [K, D], F32, kind="ExternalInput").ap(),
        nc.dram_tensor("value", [K, DV], F32, kind="ExternalInput").ap(),
        nc.dram_tensor("Wq", [D, H], F32, kind="ExternalInput").ap(),
        nc.dram_tensor("bq", [H], F32, kind="ExternalInput").ap(),
        nc.dram_tensor("Wk", [D, H], F32, kind="ExternalInput").ap(),
        nc.dram_tensor("bk", [H], F32, kind="ExternalInput").ap(),
        nc.dram_tensor("wv", [H], F32, kind="ExternalInput").ap(),
    ]
    outs = [
        nc.dram_tensor("context", [QS, DV], F32, kind="ExternalOutput").ap(),
        nc.dram_tensor("attnT", [K, QS], F32, kind="ExternalOutput").ap(),
    ]
    with tile.TileContext(nc) as tc:
        _build_tile_kernel(tc, ins, outs, n_rows=n_rows)
    nc.compile()
    return nc


_NC_CACHE = None


def _get_nc():
    global _NC_CACHE
    if _NC_CACHE is None:
        _NC_CACHE = build_nc()
    return _NC_CACHE


def make_in_maps(query, key, value, Wq, bq, Wk, bk, wv):
    in_maps = []
    for c in range(N_CORES):
        b, half = c // 2, c % 2
        in_maps.append({
            "query": np.ascontiguousarray(query[b, half * QS:(half + 1) * QS, :]),
            "key": np.ascontiguousarray(key[b]),
            "value": np.ascontiguousarray(value[b]),
            "Wq": np.ascontiguousarray(Wq),
            "bq": np.ascontiguousarray(bq),
            "Wk": np.ascontiguousarray(Wk),
            "bk": np.ascontiguousarray(bk),
            "wv": np.ascontiguousarray(wv),
        })
    return in_maps


def gather_results(results):
    context = np.empty((B, Q, DV), np.float32)
    attn = np.empty((B, Q, K), np.float32)
    for c, r in enumerate(results):
        b, half = c // 2, c % 2
        context[b, half * QS:(half + 1) * QS, :] = r["context"]
        attn[b, half * QS:(half + 1) * QS, :] = np.ascontiguousarray(r["attnT"].T)
    return context, attn


def kernel(query, key, value, Wq, bq, Wk, bk, wv, bv, **run_kwargs):
    nc = _get_nc()
    in_maps = make_in_maps(
        np.asarray(query, np.float32), np.asarray(key, np.float32),
        np.asarray(value, np.float32), np.asarray(Wq, np.float32),
        np.asarray(bq, np.float32), np.asarray(Wk, np.float32),
        np.asarray(bk, np.float32), np.asarray(wv, np.float32))
    res = run_bass_kernel_spmd(nc, in_maps, core_ids=list(range(N_CORES)),
                               **run_kwargs)
    out = gather_results(res.results)
    if run_kwargs:
        return out, res
    return out


# revision 21
# speedup vs baseline: 1.3214x; 1.0017x over previous
"""Additive attention (Bahdanau) on 8 TRN2 NeuronCores.

Full-problem shapes: query [4,512,512], key/value [4,512,512],
Wq/Wk [512,256], bq/bk [256], wv [256], bv [].

  q = query @ Wq + bq                       # [B,Q,H]
  k = key @ Wk + bk                         # [B,K,H]
  score[b,q,k] = wv . tanh(q[b,q]+k[b,k])   # (+bv, dropped: softmax-invariant)
  attn = softmax(score, axis=-1)
  context = attn @ value

Sharding: data-parallel over (batch, query-half): core c handles batch c//2,
query rows (c%2)*256:(c%2+1)*256. Each core sees its full key/value batch, so
softmax is core-local; gather is pure numpy concatenation.

Per-core kernel layout: h (hidden) on partitions. For each query row r, the
scalar engine computes feat = tanh(kTp + qTp[:, r]) as one [128h, 512k]
activation per h-chunk (per-partition bias = q values), the tensor engine then
contracts with wv (feat stationary, wv the 1-column moving operand) writing a
scoreT[k-partition, r] column into PSUM. Softmax runs on the transposed scores
without any max-subtraction (|score| <= sum|wv| ~ 13, safe in fp32): exp on
the scalar engine, key-sum via ones-vector matmul over partitions, reciprocal
broadcast across partitions by a stride-0 DMA. The normalized attnT is
directly the lhsT of the context matmul. attnT is un-transposed on the host.
"""

import numpy as np

import concourse.bass as bass
import concourse.tile as tile
from concourse import bacc, mybir
from concourse.bass_utils import run_bass_kernel_spmd
from concourse.masks import make_identity

F32 = mybir.dt.float32
F16 = mybir.dt.float16

P = 128          # partitions
D = 512          # DQ = DK (projection input dim)
H = 256          # hidden dim; HC = H // P h-chunks
K = 512          # keys per batch; KC = K // P key chunks
QS = 256         # query rows per core
DV = 512         # value dim
HC, KC, DC, QT = H // P, K // P, D // P, QS // P

N_CORES = 8
B, Q = 4, 512


def _build_tile_kernel(tc, ins, outs, n_rows=QS):
    nc = tc.nc
    query, key, value, Wq, bq, Wk, bk, wv = ins
    ctx_out, attnT_out = outs

    with tc.tile_pool(name="const", bufs=1) as const, \
         tc.tile_pool(name="proj", bufs=1) as proj, \
         tc.tile_pool(name="feat", bufs=2) as featp, \
         tc.tile_pool(name="tailp", bufs=1) as tailp, \
         tc.tile_pool(name="outp", bufs=2) as outp:

        # ---- input DMAs, critical-path first: key, Wk, query, Wq -------
        k_raw = const.tile([P, KC, D], F32)
        key_r = key.rearrange("(t p) d -> p t d", p=P)
        for t in range(KC):
            nc.sync.dma_start(k_raw[:, t, :], key_r[:, t, :])
        wk_sb = const.tile([P, DC, H], F32)
        nc.sync.dma_start(wk_sb[:], Wk.rearrange("(c p) h -> p c h", p=P))
        q_raw = const.tile([P, QT, D], F32)
        query_r = query.rearrange("(t p) d -> p t d", p=P)
        for t in range(QT):
            nc.sync.dma_start(q_raw[:, t, :], query_r[:, t, :])
        wq_sb = const.tile([P, DC, H], F32)
        nc.sync.dma_start(wq_sb[:], Wq.rearrange("(c p) h -> p c h", p=P))
        bq_sb = const.tile([P, HC], F32)
        nc.sync.dma_start(bq_sb[:], bq.rearrange("(o p) -> p o", p=P))
        bk_sb = const.tile([P, HC], F32)
        nc.sync.dma_start(bk_sb[:], bk.rearrange("(o p) -> p o", p=P))
        wv32 = const.tile([P, HC], F32)
        nc.sync.dma_start(wv32[:], wv.rearrange("(o p) -> p o", p=P))
        v_sb = const.tile([P, KC, DV], F32)   # only needed in the tail
        nc.sync.dma_start(v_sb[:], value.rearrange("(c p) v -> p c v", p=P))

        wv16 = const.tile([P, HC], F16)
        nc.vector.tensor_copy(wv16[:], wv32[:])
        ones_sb = const.tile([P, 1], F32)     # k-sum matmul lhsT
        nc.vector.memset(ones_sb[:], 1.0)
        ones_row = const.tile([1, P], F32)    # partition-broadcast via PE
        nc.vector.memset(ones_row[:], 1.0)
        ident = const.tile([P, P], F16)
        make_identity(nc, ident[:])
        # fp16 copies for cheap transposes/projections (gpsimd: it is idle)
        k16_raw = const.tile([P, KC, D], F16)
        for t in range(KC):
            nc.vector.tensor_copy(k16_raw[:, t, :], k_raw[:, t, :])
        wk16 = const.tile([P, DC, H], F16)
        nc.gpsimd.tensor_copy(wk16[:], wk_sb[:])
        q16_raw = const.tile([P, QT, D], F16)
        for t in range(QT):
            nc.vector.tensor_copy(q16_raw[:, t, :], q_raw[:, t, :])
        wq16 = const.tile([P, DC, H], F16)
        nc.gpsimd.tensor_copy(wq16[:], wq_sb[:])
        v16 = const.tile([P, KC, DV], F16)    # fp16 rhs for context matmul
        nc.gpsimd.tensor_copy(v16[:], v_sb[:])

        # ---- transpose query/key so d sits on partitions (fp16) --------
        qT = proj.tile([P, DC, QS], F16)      # [d_inner, d_chunk, q]
        kT = proj.tile([P, DC, K], F16)
        qTp = proj.tile([P, HC, QS], F32)     # fp32: feeds tensor_scalar adds
        kTp16 = proj.tile([P, HC, K], F16)
        with tc.tile_pool(name="ps_mm", bufs=2, space="PSUM") as ps_mm:
            for t in range(KC):
                for c in range(DC):
                    pst = ps_mm.tile([P, P], F16, tag="tp")
                    nc.tensor.transpose(pst[:], k16_raw[:, t, c * P:(c + 1) * P],
                                        ident[:])
                    nc.vector.tensor_copy(kT[:, c, t * P:(t + 1) * P], pst[:])
            for t in range(QT):
                for c in range(DC):
                    pst = ps_mm.tile([P, P], F16, tag="tp")
                    nc.tensor.transpose(pst[:], q16_raw[:, t, c * P:(c + 1) * P],
                                        ident[:])
                    nc.vector.tensor_copy(qT[:, c, t * P:(t + 1) * P], pst[:])

            # ---- projections, already transposed: [h, q] and [h, k] ----
            for hs in range(HC):
                psk = ps_mm.tile([P, K], F32, tag="psk", bufs=1)
                for c in range(DC):
                    nc.tensor.matmul(psk[:], wk16[:, c, hs * P:(hs + 1) * P],
                                     kT[:, c, :], start=(c == 0), stop=(c == DC - 1))
                nc.vector.tensor_scalar_add(kTp16[:, hs, :], psk[:],
                                            bk_sb[:, hs:hs + 1])
                psq = ps_mm.tile([P, QS], F32, tag="psq", bufs=1)
                for c in range(DC):
                    nc.tensor.matmul(psq[:], wq16[:, c, hs * P:(hs + 1) * P],
                                     qT[:, c, :], start=(c == 0), stop=(c == DC - 1))
                nc.vector.tensor_scalar_add(qTp[:, hs, :], psq[:],
                                            bq_sb[:, hs:hs + 1])

        # ---- main loop: per 8-row group, DVE adds -> one big tanh ->
        #      per-row wv matvecs into transposed-score PSUM columns -----
        G = 8
        n_groups = (n_rows + G - 1) // G
        with tc.tile_pool(name="ps_score", bufs=1, space="PSUM") as ps_score, \
             tc.tile_pool(name="ps_tail", bufs=1, space="PSUM") as ps_tail, \
             tc.tile_pool(name="sump", bufs=2) as sump:
            score_ps = [ps_score.tile([P, HC, QS], F32, name=f"score_{kc}")
                        for kc in range(KC)]
            ssum = tailp.tile([P, KC, QS], F32)
            expT = tailp.tile([P, KC, QS], F32)
            sums_ps = ps_tail.tile([P, QS], F32, tag="sums")
            attnT = tailp.tile([P, KC, QS], F32)
            recipB = tailp.tile([P, QS], F32)

            def tail_half(t):
                """softmax + context for query columns [t*P, (t+1)*P)."""
                cs = slice(t * P, (t + 1) * P)
                for kc in range(KC):
                    nc.vector.tensor_reduce(
                        ssum[:, kc, cs],
                        score_ps[kc][:, :, cs].rearrange("p h c -> p c h"),
                        axis=mybir.AxisListType.X, op=mybir.AluOpType.add)
                for kc in range(KC):
                    nc.scalar.activation(expT[:, kc, cs], ssum[:, kc, cs],
                                         mybir.ActivationFunctionType.Exp)
                for kc in range(KC):
                    nc.tensor.matmul(sums_ps[0:1, cs], ones_sb[:],
                                     expT[:, kc, cs],
                                     start=(kc == 0), stop=(kc == KC - 1))
                sums_sb = tailp.tile([1, P], F32, tag="sums_sb", bufs=2,
                                     name="sums_sb")
                nc.vector.tensor_copy(sums_sb[:], sums_ps[0:1, cs])
                # broadcast row across partitions via rank-1 PE outer product
                bc_ps = ps_tail.tile([P, P], F32, tag="bc", bufs=1)
                nc.tensor.matmul(bc_ps[:], ones_row[:], sums_sb[:],
                                 start=True, stop=True)
                nc.vector.reciprocal(recipB[:, cs], bc_ps[:])
                nc.vector.tensor_tensor(
                    attnT[:, :, cs], expT[:, :, cs],
                    recipB[:, None, cs].to_broadcast((P, KC, P)),
                    mybir.AluOpType.mult)
                attnT16 = tailp.tile([P, KC, P], F16, tag="attnT16", bufs=2,
                                     name="attnT16")
                nc.vector.tensor_copy(attnT16[:], attnT[:, :, cs])
                nc.sync.dma_start(
                    attnT_out.rearrange("(c p) q -> p c q", p=P)[:, :, cs],
                    attnT[:, :, cs])
                psc = ps_tail.tile([P, DV], F32, tag="ctx", bufs=1)
                for kc in range(KC):
                    nc.tensor.matmul(psc[:], attnT16[:, kc, :], v16[:, kc, :],
                                     start=(kc == 0), stop=(kc == KC - 1))
                ctx_sb = outp.tile([P, DV], F32, tag="ctx_sb")
                nc.vector.tensor_copy(ctx_sb[:], psc[:])
                nc.sync.dma_start(
                    ctx_out.rearrange("(t p) v -> p t v", p=P)[:, t, :],
                    ctx_sb[:])

            for g in range(n_groups):
                rows = range(g * G, min((g + 1) * G, n_rows))
                for hs in range(HC):
                    sums = sump.tile([P, G, K], F16, tag="sums")
                    for j, r in enumerate(rows):
                        nc.vector.tensor_scalar_add(
                            sums[:, j, :], kTp16[:, hs, :], qTp[:, hs, r:r + 1])
                    feat = featp.tile([P, G, K], F16, tag="feat")
                    nc.scalar.activation(feat[:], sums[:],
                                         mybir.ActivationFunctionType.Tanh)
                    for j, r in enumerate(rows):
                        for kc in range(KC):
                            nc.tensor.matmul(
                                score_ps[kc][:, hs, r:r + 1],
                                feat[:, j, kc * P:(kc + 1) * P],
                                wv16[:, hs:hs + 1],
                                start=True, stop=True)
                if n_rows == QS and (g + 1) * G == QS // 2:
                    tail_half(0)
            tail_half(1)
            if n_rows < QS:
                tail_half(0)


def build_nc(n_rows=QS):
    nc = bacc.Bacc("TRN2", target_bir_lowering=False, debug=False)
    ins = [
        nc.dram_tensor("query", [QS, D], F32, kind="ExternalInput").ap(),
        nc.dram_tensor("key", [K, D], F32, kind="ExternalInput").ap(),
        nc.dram_tensor("value", [K, DV], F32, kind="ExternalInput").ap(),
        nc.dram_tensor("Wq", [D, H], F32, kind="ExternalInput").ap(),
        nc.dram_tensor("bq", [H], F32, kind="ExternalInput").ap(),
        nc.dram_tensor("Wk", [D, H], F32, kind="ExternalInput").ap(),
        nc.dram_tensor("bk", [H], F32, kind="ExternalInput").ap(),
        nc.dram_tensor("wv", [H], F32, kind="ExternalInput").ap(),
    ]
    outs = [
        nc.dram_tensor("context", [QS, DV], F32, kind="ExternalOutput").ap(),
        nc.dram_tensor("attnT", [K, QS], F32, kind="ExternalOutput").ap(),
    ]
    with tile.TileContext(nc) as tc:
        _build_tile_kernel(tc, ins, outs, n_rows=n_rows)
    nc.compile()
    return nc


_NC_CACHE = None


def _get_nc():
    global _NC_CACHE
    if _NC_CACHE is None:
        _NC_CACHE = build_nc()
    return _NC_CACHE


def make_in_maps(query, key, value, Wq, bq, Wk, bk, wv):
    in_maps = []
    for c in range(N_CORES):
        b, half = c // 2, c % 2
        in_maps.append({
            "query": np.ascontiguousarray(query[b, half * QS:(half + 1) * QS, :]),
            "key": np.ascontiguousarray(key[b]),
            "value": np.ascontiguousarray(value[b]),
            "Wq": np.ascontiguousarray(Wq),
            "bq": np.ascontiguousarray(bq),
            "Wk": np.ascontiguousarray(Wk),
            "bk": np.ascontiguousarray(bk),
            "wv": np.ascontiguousarray(wv),
        })
    return in_maps


def gather_results(results):
    context = np.empty((B, Q, DV), np.float32)
    attn = np.empty((B, Q, K), np.float32)
    for c, r in enumerate(results):
        b, half = c // 2, c % 2
        context[b, half * QS:(half + 1) * QS, :] = r["context"]
        attn[b, half * QS:(half + 1) * QS, :] = np.ascontiguousarray(r["attnT"].T)
    return context, attn


def kernel(query, key, value, Wq, bq, Wk, bk, wv, bv, **run_kwargs):
    nc = _get_nc()
    in_maps = make_in_maps(
        np.asarray(query, np.float32), np.asarray(key, np.float32),
        np.asarray(value, np.float32), np.asarray(Wq, np.float32),
        np.asarray(bq, np.float32), np.asarray(Wk, np.float32),
        np.asarray(bk, np.float32), np.asarray(wv, np.float32))
    res = run_bass_kernel_spmd(nc, in_maps, core_ids=list(range(N_CORES)),
                               **run_kwargs)
    out = gather_results(res.results)
    if run_kwargs:
        return out, res
    return out
